# revision 17
# baseline (speedup 1.0000x reference)
"""Trainium2 Bass kernel for a single-step attention-GRU decoder.

Tensor-parallel over 8 NeuronCores: the vocab dimension of the output
projection is sharded (6400 rows/core of a 51200-row padded W_out); the
hidden-dim weights are sharded where cheap (W_comb rows, w_ih/w_hh
columns) and the tiny attention phase is replicated. Two collectives:
AllReduce of the partial GRU gate pre-activations, AllGather of the
per-core (max, sumexp) log-softmax statistics.

All GEMVs run on the vector engine as a fused multiply+reduce
(scalar_tensor_tensor with accum_out) over weight tiles laid out
[128 out-rows, K free] so every weight byte streams from HBM fully
contiguously.
"""

import os
import sys
import types

import numpy as np

VOCAB, H, MAXLEN = 50257, 1024, 128
N_CORES = 8
VPAD = 51200  # 8 * 6400
SHARD = VPAD // N_CORES  # 6400 vocab rows per core
VTILES = SHARD // 128  # 50 [128, H] tiles per core
NEG_BIG = -1.0e30


def _install_ntff_hook():
    """Make antenv.axon_hooks importable (the RL image ships a stub antenv
    without it) and register the ctypes NTFF hook so trace=True works."""
    if "antenv.axon_hooks" in sys.modules:
        return
    try:
        import antenv
    except ImportError:
        return
    mod = types.ModuleType("antenv.axon_hooks")
    mod._hook = None

    def set_axon_ntff_profile_hook(h):
        mod._hook = h

    def get_axon_ntff_profile_hook():
        return mod._hook

    mod.set_axon_ntff_profile_hook = set_axon_ntff_profile_hook
    mod.get_axon_ntff_profile_hook = get_axon_ntff_profile_hook
    sys.modules["antenv.axon_hooks"] = mod
    antenv.axon_hooks = mod
    try:
        from trn_agent_boot.trn_boot import _ntff_profile_via_ctypes

        hook = _ntff_profile_via_ctypes("/opt/axon/libaxon_pjrt.so")
        if hook is not None:
            set_axon_ntff_profile_hook(hook)
    except Exception:
        pass


_install_ntff_hook()

import concourse.bacc as bacc
import concourse.mybir as mybir
import concourse.tile as tile
from concourse.bass_utils import run_bass_kernel_spmd

F32 = mybir.dt.float32
AX = mybir.AxisListType
ALU = mybir.AluOpType
ACTF = mybir.ActivationFunctionType


def _build(stage=5):
    # stage 1: W_out stream + GEMV + transpose/store (hb from h0)
    # stage 2: + local log-softmax stats (no collectives)
    # stage 3: + stats AllGather
    # stage 4: + phase A attention
    # stage 5: full (+ phase B GRU + AllReduce)
    nc = bacc.Bacc("TRN2", target_bir_lowering=False, debug=False,
                   num_devices=N_CORES)

    # ---- kernel I/O (per core) ----
    e_d = nc.dram_tensor("e_vec", [1, H], F32, kind="ExternalInput")
    h0_d = nc.dram_tensor("h0_vec", [1, H], F32, kind="ExternalInput")
    h0c_d = nc.dram_tensor("h0c_row", [1, 128], F32, kind="ExternalInput")
    enc_d = nc.dram_tensor("enc", [MAXLEN, H], F32, kind="ExternalInput")
    wa_d = nc.dram_tensor("wa", [MAXLEN, 2 * H], F32, kind="ExternalInput")
    bac_d = nc.dram_tensor("ba_row", [1, 128], F32, kind="ExternalInput")
    wc_d = nc.dram_tensor("wc", [128, 2 * H], F32, kind="ExternalInput")
    bcc_d = nc.dram_tensor("bc_row", [1, 128], F32, kind="ExternalInput")
    wih_d = nc.dram_tensor("wihT", [128, 3 * H], F32, kind="ExternalInput")
    whh_d = nc.dram_tensor("whhT", [128, 3 * H], F32, kind="ExternalInput")
    gb_d = nc.dram_tensor("gbias", [1, 4 * H], F32, kind="ExternalInput")
    wo_d = nc.dram_tensor("wo", [SHARD, H], F32, kind="ExternalInput")
    bo_d = nc.dram_tensor("bo", [128, VTILES], F32, kind="ExternalInput")

    out_d = nc.dram_tensor("out_shard", [VTILES, 128], F32, kind="ExternalOutput")
    hn_d = nc.dram_tensor("h_new", [1, H], F32, kind="ExternalOutput")
    aw_d = nc.dram_tensor("attn_w", [1, MAXLEN], F32, kind="ExternalOutput")

    eye_d = nc.inline_tensor(np.eye(128, dtype=np.float32), name="eye128")
    ones_d = nc.inline_tensor(np.ones((1, 128), dtype=np.float32), name="ones_row")

    with tile.TileContext(nc) as tc:
        with (
            tc.tile_pool(name="wts", bufs=1) as wts,
            tc.tile_pool(name="small", bufs=1) as small,
            tc.tile_pool(name="stream", bufs=18) as stream,
            tc.tile_pool(name="scrc", bufs=2) as scrc,
            tc.tile_pool(name="psum", bufs=2, space="PSUM") as psum,
            tc.tile_pool(name="dram", bufs=1, space="DRAM") as dram,
        ):
            def ps_pair():
                return psum.tile([128, 1024], F32, tag="pair", name="ps_pair")

            def ps_one(p, f):
                return psum.tile([p, f], F32, tag="bank1", name="ps_one")

            def gemv(w_ap, x_ap, accum_ap, scr_ap):
                # accum[p] = sum_k w[p, k] * x[p, k] (single DVE pass)
                nc.vector.scalar_tensor_tensor(
                    out=scr_ap, in0=w_ap, scalar=1.0, in1=x_ap,
                    op0=ALU.bypass, op1=ALU.mult, accum_out=accum_ap)

            # ---- tiny rows on gpsimd ring; weights on sync ring ----
            ones = wts.tile([1, 128], F32)
            nc.gpsimd.dma_start(ones[:], ones_d.ap())
            cat = small.tile([1, 2 * H], F32, tag="catrow")
            nc.gpsimd.dma_start(cat[:, 0:H], e_d.ap())
            nc.gpsimd.dma_start(cat[:, H : 2 * H], h0_d.ap())
            bacr = wts.tile([1, 128], F32)
            nc.gpsimd.dma_start(bacr[:], bac_d.ap())
            h0row = small.tile([1, H], F32)
            nc.gpsimd.dma_start(h0row[:], h0_d.ap())
            h0cr = wts.tile([1, 128], F32)
            nc.gpsimd.dma_start(h0cr[:], h0c_d.ap())
            bccr = wts.tile([1, 128], F32)
            nc.gpsimd.dma_start(bccr[:], bcc_d.ap())
            # transpose the three rows to columns on chip (PE is idle)
            bac = wts.tile([128, 1], F32)
            ps = ps_one(128, 1)
            nc.tensor.matmul(ps[:], bacr[:], ones[:, 0:1])
            nc.scalar.copy(bac[:], ps[:])
            h0col = wts.tile([128, 1], F32)
            ps = ps_one(128, 1)
            nc.tensor.matmul(ps[:], h0cr[:], ones[:, 0:1])
            nc.scalar.copy(h0col[:], ps[:])
            bcc = wts.tile([128, 1], F32)
            ps = ps_one(128, 1)
            nc.tensor.matmul(ps[:], bccr[:], ones[:, 0:1])
            nc.scalar.copy(bcc[:], ps[:])
            # ones column [8,1] for the cross-core partial reduction
            ones8 = wts.tile([8, 1], F32)
            ps = ps_one(8, 1)
            nc.tensor.matmul(ps[:], ones[:, 0:8], ones[:, 0:1])
            nc.scalar.copy(ones8[:], ps[:])
            # pre-fill the AllReduce input with the GRU biases (DRAM->DRAM);
            # the partials are later accumulated on top via DMA accum_op.
            cc_in = dram.tile([1, 4 * H], F32)
            nc.gpsimd.dma_start(cc_in[:], gb_d.ap())
            # preload ACT tables off the critical path (Exp/Sigmoid/Tanh/Ln)
            warm = small.tile([1, 4], F32, name="warm")
            nc.scalar.activation(warm[:, 0:1], ones[:, 0:1], ACTF.Exp)
            nc.scalar.activation(warm[:, 1:2], ones[:, 0:1], ACTF.Sigmoid)
            nc.scalar.activation(warm[:, 2:3], ones[:, 0:1], ACTF.Tanh)
            nc.scalar.activation(warm[:, 3:4], ones[:, 0:1], ACTF.Ln)
            # weights on the sync (HWDGE) ring, ahead of the W_out stream
            eye = wts.tile([128, 128], F32)
            nc.sync.dma_start(eye[:], eye_d.ap())
            wa = wts.tile([128, 2 * H], F32)
            nc.sync.dma_start(wa[:], wa_d.ap())
            enc = wts.tile([MAXLEN, H], F32)
            nc.sync.dma_start(enc[:], enc_d.ap())
            wc = wts.tile([128, 2 * H], F32)
            nc.sync.dma_start(wc[:], wc_d.ap())
            wih = wts.tile([128, 3 * H], F32)
            nc.sync.dma_start(wih[:], wih_d.ap())
            whh = wts.tile([128, 3 * H], F32)
            nc.sync.dma_start(whh[:], whh_d.ap())
            bo = wts.tile([128, VTILES], F32)
            nc.sync.dma_start(bo[:], bo_d.ap())

            # ---- W_out stream (big; behind the small loads) ----
            wo_tiles = []
            for t in range(VTILES):
                wt = stream.tile([128, H], F32, tag="wo_t", name="wo_t")
                nc.sync.dma_start(wt[:], wo_d.ap()[128 * t : 128 * t + 128, :])
                wo_tiles.append(wt)

            # ---- phase A: attention (replicated on every core) ----
            if stage < 4:
                hb = small.tile([128, H], F32)
                nc.gpsimd.partition_broadcast(hb[:], h0row[:])
                zrow = small.tile([1, MAXLEN], F32)
                nc.vector.memset(zrow[:], 0.0)
                nc.gpsimd.dma_start(aw_d.ap(), zrow[:])
                nc.gpsimd.dma_start(hn_d.ap(), h0row[:])
            if stage >= 4:
                # broadcast cat=[e,h0] to 128 partitions
                catb = small.tile([128, 2 * H], F32, tag="catb")
                nc.gpsimd.partition_broadcast(catb[:], cat[:])

                # attn logits[p] = sum_k Wa[p,k]*cat[k] + ba[p]
                scr_a = small.tile([128, 2 * H], F32, tag="scrAB", name="scr_a")
                alog = small.tile([128, 1], F32)
                gemv(wa[:], catb[:], alog[:], scr_a[:])
                nc.vector.tensor_tensor(alog[:], alog[:], bac[:], op=ALU.add)
                # transpose to [1,128]
                alog_ps = ps_one(1, 128)
                nc.tensor.matmul(alog_ps[:], alog[:], eye[:])
                # softmax over free axis (logits are O(1): skip max-sub)
                aexp = small.tile([1, 128], F32)
                asum = small.tile([1, 1], F32)
                nc.scalar.activation(aexp[:], alog_ps[:], ACTF.Exp,
                                     accum_out=asum[:])
                ainv = small.tile([1, 1], F32)
                nc.vector.reciprocal(ainv[:], asum[:])
                awr = small.tile([1, 128], F32)
                nc.vector.tensor_scalar_mul(awr[:], aexp[:], ainv[:])
                nc.gpsimd.dma_start(aw_d.ap(), awr[:])
                # attn_w as a column for PE
                awT_ps = ps_one(128, 1)
                nc.tensor.matmul(awT_ps[:], awr[:], ones[:, 0:1])
                awcol = small.tile([128, 1], F32)
                nc.scalar.copy(awcol[:], awT_ps[:])
                # attn_applied = attn_w @ enc -> [1, H], written into cat2 tail
                cat2 = small.tile([1, 2 * H], F32, tag="catrow")
                nc.gpsimd.dma_start(cat2[:, 0:H], e_d.ap())
                for s in range(0, H, 512):
                    ps = ps_one(1, 512)
                    nc.tensor.matmul(ps[:], awcol[:], enc[:, s : s + 512])
                    nc.scalar.copy(cat2[:, H + s : H + s + 512], ps[:])

                if stage == 4:
                    hb = small.tile([128, H], F32)
                    nc.gpsimd.partition_broadcast(hb[:], h0row[:])
                    nc.gpsimd.dma_start(hn_d.ap(), h0row[:])
            if stage >= 5:
                # ---- phase B: combine + GRU ----
                catb2 = small.tile([128, 2 * H], F32, tag="catb")
                nc.gpsimd.partition_broadcast(catb2[:], cat2[:])

                gpre = small.tile([128, 1], F32)
                scr_b = small.tile([128, 2 * H], F32, tag="scrAB", name="scr_b")
                gemv(wc[:], catb2[:], gpre[:], scr_b[:])
                nc.vector.tensor_tensor(gpre[:], gpre[:], bcc[:], op=ALU.add)
                gcol = small.tile([128, 1], F32)
                nc.scalar.activation(gcol[:], gpre[:], ACTF.Relu)

                # GRU partials into ccv = [rz_sum (2H) | i_n (H) | h_n (H)];
                # gbias is already sitting in cc_in, partials accumulate onto it
                ccv = small.tile([1, 4 * H], F32, tag="ccrow")
                for s in range(4):
                    ps = ps_one(1, 512)
                    o = s * 512
                    nc.tensor.matmul(ps[:], gcol[:], wih[:, o : o + 512],
                                     start=True, stop=False)
                    nc.tensor.matmul(ps[:], h0col[:], whh[:, o : o + 512],
                                     start=False, stop=True)
                    nc.scalar.copy(ccv[:, o : o + 512], ps[:])
                for s in range(2):
                    ps = ps_one(1, 512)
                    o = 2 * H + s * 512
                    nc.tensor.matmul(ps[:], gcol[:], wih[:, o : o + 512])
                    nc.scalar.copy(ccv[:, o : o + 512], ps[:])
                for s in range(2):
                    ps = ps_one(1, 512)
                    o = 3 * H + s * 512
                    nc.tensor.matmul(ps[:], h0col[:],
                                     whh[:, 2 * H + s * 512 : 2 * H + s * 512 + 512])
                    nc.scalar.copy(ccv[:, o : o + 512], ps[:])

                cc_out = dram.tile([N_CORES, 4 * H], F32, addr_space="Shared")
                nc.gpsimd.dma_start(cc_in[:], ccv[:], accum_op=ALU.add)
                nc.gpsimd.collective_compute(
                    "AllGather", ALU.bypass,
                    replica_groups=[list(range(N_CORES))],
                    ins=[cc_in.opt()], outs=[cc_out.opt()],
                )
                y8 = small.tile([N_CORES, 4 * H], F32)
                nc.gpsimd.dma_start(y8[:], cc_out[:])
                # red = sum over the 8 per-core partials (PE: ones8.T @ y8)
                red = small.tile([1, 4 * H], F32, tag="ccrow")
                for s in range(0, 4 * H, 512):
                    ps = ps_one(1, 512)
                    nc.tensor.matmul(ps[:], ones8[:], y8[:, s : s + 512])
                    nc.scalar.copy(red[:, s : s + 512], ps[:])

                # gates (full H, replicated); ga ends as n, gc ends as h_new
                ga = small.tile([1, H], F32)
                nc.scalar.activation(ga[:], red[:, 0:H], ACTF.Sigmoid)  # r
                gz = small.tile([1, H], F32)
                nc.scalar.activation(gz[:], red[:, H : 2 * H], ACTF.Sigmoid)  # z
                nc.vector.tensor_tensor(ga[:], ga[:], red[:, 3 * H : 4 * H],
                                        op=ALU.mult)  # r*h_n
                nc.vector.tensor_tensor(ga[:], ga[:], red[:, 2 * H : 3 * H],
                                        op=ALU.add)  # + i_n
                nc.scalar.activation(ga[:], ga[:], ACTF.Tanh)  # n
                # h_new = n + z*(h0 - n)
                gc = small.tile([1, H], F32)
                nc.vector.tensor_tensor(gc[:], h0row[:], ga[:], op=ALU.subtract)
                nc.vector.tensor_tensor(gc[:], gz[:], gc[:], op=ALU.mult)
                hnew = gc
                nc.vector.tensor_tensor(hnew[:], ga[:], gc[:], op=ALU.add)
                nc.gpsimd.dma_start(hn_d.ap(), hnew[:])

                # broadcast h_new to 128 partitions
                hb = small.tile([128, H], F32)
                nc.gpsimd.partition_broadcast(hb[:], hnew[:])

            # ---- phase C: vocab-shard GEMV + log-softmax ----
            logits = small.tile([128, VTILES], F32)
            for t in range(VTILES):
                scr = scrc.tile([128, H], F32, tag="scrC", name="scr")
                gemv(wo_tiles[t][:], hb[:], logits[:, t : t + 1], scr[:])
            nc.vector.tensor_tensor(logits[:], logits[:], bo[:], op=ALU.add)

            if stage < 2:
                oT_ps0 = ps_one(VTILES, 128)
                nc.tensor.matmul(oT_ps0[:], logits[:], eye[:])
                oT0 = small.tile([VTILES, 128], F32, tag="oT")
                nc.scalar.copy(oT0[:], oT_ps0[:])
                nc.gpsimd.dma_start(out_d.ap(), oT0[:])
            if stage >= 2:
                mcol = small.tile([128, 1], F32)
                nc.vector.reduce_max(mcol[:], logits[:], axis=AX.X)
                nmcol = small.tile([128, 1], F32)
                nc.vector.tensor_scalar_mul(nmcol[:], mcol[:], -1.0)
                escr = small.tile([128, VTILES], F32)
                scol = small.tile([128, 1], F32)
                nc.scalar.activation(escr[:], logits[:], ACTF.Exp, bias=nmcol[:],
                                     accum_out=scol[:])
                # per-core (max, sumexp) scalars: transpose cols to rows, reduce
                mrow_ps = ps_one(1, 128)
                nc.tensor.matmul(mrow_ps[:], mcol[:], eye[:])
                srow_ps = ps_one(1, 128)
                nc.tensor.matmul(srow_ps[:], scol[:], eye[:])
                srow = small.tile([1, 128], F32)
                nc.scalar.copy(srow[:], srow_ps[:])
                mloc = small.tile([1, 1], F32)
                nc.vector.reduce_max(mloc[:], mrow_ps[:], axis=AX.X)
                nmloc = small.tile([1, 1], F32)
                nc.vector.tensor_scalar_mul(nmloc[:], mloc[:], -1.0)
                emrow = small.tile([1, 128], F32)
                nc.scalar.activation(emrow[:], mrow_ps[:], ACTF.Exp, bias=nmloc[:])
                sscr = small.tile([1, 128], F32)
                sloc = small.tile([1, 1], F32)
                nc.vector.scalar_tensor_tensor(
                    out=sscr[:], in0=emrow[:], scalar=1.0, in1=srow[:],
                    op0=ALU.bypass, op1=ALU.mult, accum_out=sloc[:])

                if stage >= 3:
                    pk = small.tile([1, 16], F32)
                    nc.vector.memset(pk[:], 0.0)
                    nc.vector.tensor_copy(pk[:, 0:1], mloc[:])
                    nc.vector.tensor_copy(pk[:, 1:2], sloc[:])

                    st_in = dram.tile([1, 16], F32)
                    st_out = dram.tile([N_CORES, 16], F32, addr_space="Shared")
                    nc.gpsimd.dma_start(st_in[:], pk[:])
                    nc.gpsimd.collective_compute(
                        "AllGather", ALU.bypass,
                        replica_groups=[list(range(N_CORES))],
                        ins=[st_in.opt()], outs=[st_out.opt()],
                    )
                    stats = small.tile([1, 16 * N_CORES], F32)
                    nc.gpsimd.dma_start(stats[:], st_out[:])

                    sv = stats.rearrange("p (a b) -> p a b", b=16)
                    m8 = sv[:, :, 0:1].rearrange("p a b -> p (a b)")
                    s8 = sv[:, :, 1:2].rearrange("p a b -> p (a b)")
                    gm = small.tile([1, 1], F32)
                    nc.vector.reduce_max(gm[:], m8, axis=AX.X)
                    ngm = small.tile([1, 1], F32)
                    nc.vector.tensor_scalar_mul(ngm[:], gm[:], -1.0)
                    e8 = small.tile([1, N_CORES], F32)
                    nc.scalar.activation(e8[:], m8, ACTF.Exp, bias=ngm[:])
                    s8scr = small.tile([1, N_CORES], F32)
                    gs = small.tile([1, 1], F32)
                    nc.vector.scalar_tensor_tensor(
                        out=s8scr[:], in0=e8[:], scalar=1.0, in1=s8,
                        op0=ALU.bypass, op1=ALU.mult, accum_out=gs[:])
                else:
                    gm, gs = mloc, sloc
                lns = small.tile([1, 1], F32)
                nc.scalar.activation(lns[:], gs[:], ACTF.Ln)
                csum = small.tile([1, 1], F32)
                nc.vector.tensor_tensor(csum[:], gm[:], lns[:], op=ALU.add)
                ncs = small.tile([1, 1], F32)
                nc.vector.tensor_scalar_mul(ncs[:], csum[:], -1.0)
                # broadcast -C to 128 partitions
                cb_ps = ps_one(128, 1)
                nc.tensor.matmul(cb_ps[:], ones[:], ncs[:])
                cb = small.tile([128, 1], F32)
                nc.scalar.copy(cb[:], cb_ps[:])
                # out = logits - C (in place)
                nc.vector.tensor_scalar_add(logits[:], logits[:], cb[:])
                # transpose [128, VTILES] -> [VTILES, 128] for a contiguous store
                oT_ps = ps_one(VTILES, 128)
                nc.tensor.matmul(oT_ps[:], logits[:], eye[:])
                oT = small.tile([VTILES, 128], F32, tag="oT")
                nc.scalar.copy(oT[:], oT_ps[:])
                nc.gpsimd.dma_start(out_d.ap(), oT[:])

    nc.compile()
    return nc


_NC = None


def _get_nc():
    global _NC
    if _NC is None:
        _NC = _build(stage=int(os.environ.get("BASS_STAGE", "5")))
    return _NC


def _prep_in_maps(x, h, encoder_outputs, emb, W_attn, b_attn, W_comb, b_comb,
                  w_ih, w_hh, b_ih, b_hh, W_out, b_out):
    e = np.ascontiguousarray(
        emb[int(np.asarray(x).ravel()[0])], dtype=np.float32).reshape(1, H)
    h0 = np.ascontiguousarray(h, dtype=np.float32).reshape(1, H)
    enc = np.ascontiguousarray(encoder_outputs, dtype=np.float32)
    W_attn = np.ascontiguousarray(W_attn, dtype=np.float32)
    ba_col = np.ascontiguousarray(b_attn, dtype=np.float32).reshape(128, 1)
    b_ih = np.asarray(b_ih, dtype=np.float32)
    b_hh = np.asarray(b_hh, dtype=np.float32)
    gbias = np.concatenate([
        b_ih[0:H] + b_hh[0:H],
        b_ih[H : 2 * H] + b_hh[H : 2 * H],
        b_ih[2 * H : 3 * H],
        b_hh[2 * H : 3 * H],
    ]).astype(np.float32).reshape(1, 4 * H)
    zeros_gb = np.zeros((1, 4 * H), np.float32)

    in_maps = []
    for j in range(N_CORES):
        rows = slice(128 * j, 128 * (j + 1))
        wihT = np.ascontiguousarray(w_ih[:, rows].T, dtype=np.float32)
        whhT = np.ascontiguousarray(w_hh[:, rows].T, dtype=np.float32)
        r0 = SHARD * j
        r1 = min(SHARD * (j + 1), VOCAB)
        wo = np.asarray(W_out[r0:r1], dtype=np.float32)
        bov = np.asarray(b_out[r0:r1], dtype=np.float32)
        if wo.shape[0] < SHARD:
            wo = np.concatenate(
                [wo, np.zeros((SHARD - wo.shape[0], H), np.float32)])
            bov = np.concatenate(
                [bov, np.full((SHARD - bov.shape[0],), NEG_BIG, np.float32)])
        in_maps.append({
            "e_vec": e,
            "h0_vec": h0,
            "h0c_row": np.ascontiguousarray(h0[0, rows]).reshape(1, 128),
            "enc": enc,
            "wa": W_attn,
            "ba_row": ba_col.reshape(1, 128),
            "wc": np.ascontiguousarray(W_comb[rows], dtype=np.float32),
            "bc_row": np.ascontiguousarray(
                b_comb[rows], dtype=np.float32).reshape(1, 128),
            "wihT": wihT,
            "whhT": whhT,
            "gbias": gbias if j == 0 else zeros_gb,
            "wo": np.ascontiguousarray(wo),
            "bo": np.ascontiguousarray(bov.reshape(VTILES, 128).T),
        })
    return in_maps


def kernel(x, h, encoder_outputs, emb, W_attn, b_attn, W_comb, b_comb,
           w_ih, w_hh, b_ih, b_hh, W_out, b_out, _trace=False):
    in_maps = _prep_in_maps(x, h, encoder_outputs, emb, W_attn, b_attn,
                            W_comb, b_comb, w_ih, w_hh, b_ih, b_hh,
                            W_out, b_out)
    nc = _get_nc()
    kw = {"tmpdir": "/root/problem/profdir"} if _trace else {}
    res = run_bass_kernel_spmd(nc, in_maps, core_ids=list(range(N_CORES)),
                               trace=_trace, **kw)
    out = np.concatenate(
        [res.results[j]["out_shard"].reshape(-1) for j in range(N_CORES)]
    )[:VOCAB].reshape(1, VOCAB)
    h_new = res.results[0]["h_new"].reshape(1, 1, H)
    attn_w = res.results[0]["attn_w"].reshape(1, MAXLEN)
    if _trace:
        return (out, h_new, attn_w), res
    return out, h_new, attn_w


# revision 18
# speedup vs baseline: 1.2355x; 1.2355x over previous
"""Trainium2 Bass kernel for a single-step attention-GRU decoder.

Tensor-parallel over 8 NeuronCores: the vocab dimension of the output
projection is sharded (6400 rows/core of a 51200-row padded W_out); the
hidden-dim weights are sharded where cheap (W_comb rows, w_ih/w_hh
columns) and the tiny attention phase is replicated. Two collectives:
AllReduce of the partial GRU gate pre-activations, AllGather of the
per-core (max, sumexp) log-softmax statistics.

All GEMVs run on the vector engine as a fused multiply+reduce
(scalar_tensor_tensor with accum_out) over weight tiles laid out
[128 out-rows, K free] so every weight byte streams from HBM fully
contiguously.
"""

import os
import sys
import types

import numpy as np

VOCAB, H, MAXLEN = 50257, 1024, 128
N_CORES = 8
VPAD = 51200  # 8 * 6400
SHARD = VPAD // N_CORES  # 6400 vocab rows per core
VTILES = SHARD // 128  # 50 [128, H] tiles per core
NEG_BIG = -1.0e30


def _install_ntff_hook():
    """Make antenv.axon_hooks importable (the RL image ships a stub antenv
    without it) and register the ctypes NTFF hook so trace=True works."""
    if "antenv.axon_hooks" in sys.modules:
        return
    try:
        import antenv
    except ImportError:
        return
    mod = types.ModuleType("antenv.axon_hooks")
    mod._hook = None

    def set_axon_ntff_profile_hook(h):
        mod._hook = h

    def get_axon_ntff_profile_hook():
        return mod._hook

    mod.set_axon_ntff_profile_hook = set_axon_ntff_profile_hook
    mod.get_axon_ntff_profile_hook = get_axon_ntff_profile_hook
    sys.modules["antenv.axon_hooks"] = mod
    antenv.axon_hooks = mod
    try:
        from trn_agent_boot.trn_boot import _ntff_profile_via_ctypes

        hook = _ntff_profile_via_ctypes("/opt/axon/libaxon_pjrt.so")
        if hook is not None:
            set_axon_ntff_profile_hook(hook)
    except Exception:
        pass


_install_ntff_hook()

import concourse.bacc as bacc
import concourse.mybir as mybir
import concourse.tile as tile
from concourse.bass_utils import run_bass_kernel_spmd

F32 = mybir.dt.float32
AX = mybir.AxisListType
ALU = mybir.AluOpType
ACTF = mybir.ActivationFunctionType


def _build(stage=5):
    # stage 1: W_out stream + GEMV + transpose/store (hb from h0)
    # stage 2: + local log-softmax stats (no collectives)
    # stage 3: + stats AllGather
    # stage 4: + phase A attention
    # stage 5: full (+ phase B GRU + AllReduce)
    nc = bacc.Bacc("TRN2", target_bir_lowering=False, debug=False,
                   num_devices=N_CORES)

    # ---- kernel I/O (per core) ----
    e_d = nc.dram_tensor("e_vec", [1, H], F32, kind="ExternalInput")
    h0_d = nc.dram_tensor("h0_vec", [1, H], F32, kind="ExternalInput")
    h0c_d = nc.dram_tensor("h0c_row", [1, 128], F32, kind="ExternalInput")
    enc_d = nc.dram_tensor("enc", [MAXLEN, H], F32, kind="ExternalInput")
    wa_d = nc.dram_tensor("wa", [MAXLEN, 2 * H], F32, kind="ExternalInput")
    bac_d = nc.dram_tensor("ba_row", [1, 128], F32, kind="ExternalInput")
    wc_d = nc.dram_tensor("wc", [128, 2 * H], F32, kind="ExternalInput")
    bcc_d = nc.dram_tensor("bc_row", [1, 128], F32, kind="ExternalInput")
    wih_d = nc.dram_tensor("wihT", [128, 3 * H], F32, kind="ExternalInput")
    whh_d = nc.dram_tensor("whhT", [128, 3 * H], F32, kind="ExternalInput")
    gb_d = nc.dram_tensor("gbias", [1, 4 * H], F32, kind="ExternalInput")
    wo_d = nc.dram_tensor("wo", [SHARD, H], F32, kind="ExternalInput")
    bo_d = nc.dram_tensor("bo", [128, VTILES], F32, kind="ExternalInput")

    out_d = nc.dram_tensor("out_shard", [VTILES, 128], F32, kind="ExternalOutput")
    hn_d = nc.dram_tensor("h_new", [1, H], F32, kind="ExternalOutput")
    aw_d = nc.dram_tensor("attn_w", [1, MAXLEN], F32, kind="ExternalOutput")

    eye_d = nc.inline_tensor(np.eye(128, dtype=np.float32), name="eye128")
    ones_d = nc.inline_tensor(np.ones((1, 128), dtype=np.float32), name="ones_row")

    with tile.TileContext(nc) as tc:
        with (
            tc.tile_pool(name="wts", bufs=1) as wts,
            tc.tile_pool(name="small", bufs=1) as small,
            tc.tile_pool(name="stream", bufs=20) as stream,
            tc.tile_pool(name="scrc", bufs=2) as scrc,
            tc.tile_pool(name="psum", bufs=2, space="PSUM") as psum,
            tc.tile_pool(name="dram", bufs=1, space="DRAM") as dram,
        ):
            def ps_pair():
                return psum.tile([128, 1024], F32, tag="pair", name="ps_pair")

            def ps_one(p, f):
                return psum.tile([p, f], F32, tag="bank1", name="ps_one")

            def gemv(w_ap, x_ap, accum_ap, scr_ap):
                # accum[p] = sum_k w[p, k] * x[p, k] (single DVE pass)
                nc.vector.scalar_tensor_tensor(
                    out=scr_ap, in0=w_ap, scalar=1.0, in1=x_ap,
                    op0=ALU.bypass, op1=ALU.mult, accum_out=accum_ap)

            # ---- tiny rows on gpsimd ring; weights on sync ring ----
            ones = wts.tile([1, 128], F32)
            nc.gpsimd.dma_start(ones[:], ones_d.ap())
            cat = small.tile([1, 2 * H], F32, tag="catrow")
            nc.gpsimd.dma_start(cat[:, 0:H], e_d.ap())
            nc.gpsimd.dma_start(cat[:, H : 2 * H], h0_d.ap())
            bacr = wts.tile([1, 128], F32)
            nc.gpsimd.dma_start(bacr[:], bac_d.ap())
            h0row = small.tile([1, H], F32)
            nc.gpsimd.dma_start(h0row[:], h0_d.ap())
            h0cr = wts.tile([1, 128], F32)
            nc.gpsimd.dma_start(h0cr[:], h0c_d.ap())
            bccr = wts.tile([1, 128], F32)
            nc.gpsimd.dma_start(bccr[:], bcc_d.ap())
            # transpose the three rows to columns on chip (PE is idle)
            bac = wts.tile([128, 1], F32)
            ps = ps_one(128, 1)
            nc.tensor.matmul(ps[:], bacr[:], ones[:, 0:1])
            nc.scalar.copy(bac[:], ps[:])
            h0col = wts.tile([128, 1], F32)
            ps = ps_one(128, 1)
            nc.tensor.matmul(ps[:], h0cr[:], ones[:, 0:1])
            nc.scalar.copy(h0col[:], ps[:])
            bcc = wts.tile([128, 1], F32)
            ps = ps_one(128, 1)
            nc.tensor.matmul(ps[:], bccr[:], ones[:, 0:1])
            nc.scalar.copy(bcc[:], ps[:])
            # pre-fill the AllReduce input with the GRU biases (DRAM->DRAM);
            # the partials are later accumulated on top via DMA accum_op.
            cc_in = dram.tile([1, 4 * H], F32)
            nc.gpsimd.dma_start(cc_in[:], gb_d.ap())
            # preload ACT tables off the critical path (Exp/Sigmoid/Tanh/Ln)
            warm = small.tile([1, 4], F32, name="warm")
            nc.scalar.activation(warm[:, 0:1], ones[:, 0:1], ACTF.Exp)
            nc.scalar.activation(warm[:, 1:2], ones[:, 0:1], ACTF.Sigmoid)
            nc.scalar.activation(warm[:, 2:3], ones[:, 0:1], ACTF.Tanh)
            nc.scalar.activation(warm[:, 3:4], ones[:, 0:1], ACTF.Ln)
            # weights on the sync (HWDGE) ring, ahead of the W_out stream
            eye = wts.tile([128, 128], F32)
            nc.sync.dma_start(eye[:], eye_d.ap())
            wa = wts.tile([128, 2 * H], F32)
            nc.sync.dma_start(wa[:], wa_d.ap())
            enc = wts.tile([MAXLEN, H], F32)
            nc.sync.dma_start(enc[:], enc_d.ap())
            wc = wts.tile([128, 2 * H], F32)
            nc.sync.dma_start(wc[:], wc_d.ap())
            wih = wts.tile([128, 3 * H], F32)
            nc.sync.dma_start(wih[:], wih_d.ap())
            whh = wts.tile([128, 3 * H], F32)
            nc.sync.dma_start(whh[:], whh_d.ap())
            bo = wts.tile([128, VTILES], F32)
            nc.sync.dma_start(bo[:], bo_d.ap())

            # ---- W_out stream (big; behind the small loads) ----
            wo_tiles = []
            for t in range(VTILES):
                wt = stream.tile([128, H], F32, tag="wo_t", name="wo_t")
                nc.sync.dma_start(wt[:], wo_d.ap()[128 * t : 128 * t + 128, :])
                wo_tiles.append(wt)

            # ---- phase A: attention (replicated on every core) ----
            if stage < 4:
                hb = small.tile([128, H], F32)
                nc.gpsimd.partition_broadcast(hb[:], h0row[:])
                zrow = small.tile([1, MAXLEN], F32)
                nc.vector.memset(zrow[:], 0.0)
                nc.gpsimd.dma_start(aw_d.ap(), zrow[:])
                nc.gpsimd.dma_start(hn_d.ap(), h0row[:])
            if stage >= 4:
                # broadcast cat=[e,h0] to 128 partitions
                catb = small.tile([128, 2 * H], F32, tag="catb")
                nc.gpsimd.partition_broadcast(catb[:], cat[:])

                # attn logits[p] = sum_k Wa[p,k]*cat[k] + ba[p]
                scr_a = small.tile([128, 2 * H], F32, tag="scrAB", name="scr_a")
                alog = small.tile([128, 1], F32)
                gemv(wa[:], catb[:], alog[:], scr_a[:])
                nc.vector.tensor_tensor(alog[:], alog[:], bac[:], op=ALU.add)
                # transpose to [1,128]
                alog_ps = ps_one(1, 128)
                nc.tensor.matmul(alog_ps[:], alog[:], eye[:])
                # softmax over free axis (logits are O(1): skip max-sub)
                aexp = small.tile([1, 128], F32)
                asum = small.tile([1, 1], F32)
                nc.scalar.activation(aexp[:], alog_ps[:], ACTF.Exp,
                                     accum_out=asum[:])
                ainv = small.tile([1, 1], F32)
                nc.vector.reciprocal(ainv[:], asum[:])
                awr = small.tile([1, 128], F32)
                nc.vector.tensor_scalar_mul(awr[:], aexp[:], ainv[:])
                nc.gpsimd.dma_start(aw_d.ap(), awr[:])
                # attn_w as a column for PE
                awT_ps = ps_one(128, 1)
                nc.tensor.matmul(awT_ps[:], awr[:], ones[:, 0:1])
                awcol = small.tile([128, 1], F32)
                nc.scalar.copy(awcol[:], awT_ps[:])
                # attn_applied = attn_w @ enc -> [1, H], written into cat2 tail
                cat2 = small.tile([1, 2 * H], F32, tag="catrow")
                nc.gpsimd.dma_start(cat2[:, 0:H], e_d.ap())
                for s in range(0, H, 512):
                    ps = ps_one(1, 512)
                    nc.tensor.matmul(ps[:], awcol[:], enc[:, s : s + 512])
                    nc.scalar.copy(cat2[:, H + s : H + s + 512], ps[:])

                if stage == 4:
                    hb = small.tile([128, H], F32)
                    nc.gpsimd.partition_broadcast(hb[:], h0row[:])
                    nc.gpsimd.dma_start(hn_d.ap(), h0row[:])
            if stage >= 5:
                # ---- phase B: combine + GRU ----
                catb2 = small.tile([128, 2 * H], F32, tag="catb")
                nc.gpsimd.partition_broadcast(catb2[:], cat2[:])

                gpre = small.tile([128, 1], F32)
                scr_b = small.tile([128, 2 * H], F32, tag="scrAB", name="scr_b")
                gemv(wc[:], catb2[:], gpre[:], scr_b[:])
                nc.vector.tensor_tensor(gpre[:], gpre[:], bcc[:], op=ALU.add)
                gcol = small.tile([128, 1], F32)
                nc.scalar.activation(gcol[:], gpre[:], ACTF.Relu)

                # GRU partials into ccv = [rz_sum (2H) | i_n (H) | h_n (H)];
                # gbias is already sitting in cc_in, partials accumulate onto it
                ccv = small.tile([1, 4 * H], F32, tag="ccrow")
                for s in range(4):
                    ps = ps_one(1, 512)
                    o = s * 512
                    nc.tensor.matmul(ps[:], gcol[:], wih[:, o : o + 512],
                                     start=True, stop=False)
                    nc.tensor.matmul(ps[:], h0col[:], whh[:, o : o + 512],
                                     start=False, stop=True)
                    nc.scalar.copy(ccv[:, o : o + 512], ps[:])
                for s in range(2):
                    ps = ps_one(1, 512)
                    o = 2 * H + s * 512
                    nc.tensor.matmul(ps[:], gcol[:], wih[:, o : o + 512])
                    nc.scalar.copy(ccv[:, o : o + 512], ps[:])
                for s in range(2):
                    ps = ps_one(1, 512)
                    o = 3 * H + s * 512
                    nc.tensor.matmul(ps[:], h0col[:],
                                     whh[:, 2 * H + s * 512 : 2 * H + s * 512 + 512])
                    nc.scalar.copy(ccv[:, o : o + 512], ps[:])

                cc_out = dram.tile([1, 4 * H], F32, addr_space="Shared")
                nc.gpsimd.dma_start(cc_in[:], ccv[:], accum_op=ALU.add)
                nc.gpsimd.collective_compute(
                    "AllReduce", ALU.add,
                    replica_groups=[list(range(N_CORES))],
                    ins=[cc_in.opt()], outs=[cc_out.opt()],
                )
                red = small.tile([1, 4 * H], F32, tag="ccrow")
                nc.gpsimd.dma_start(red[:], cc_out[:])

                # gates (full H, replicated); ga ends as n, gc ends as h_new
                ga = small.tile([1, H], F32)
                nc.scalar.activation(ga[:], red[:, 0:H], ACTF.Sigmoid)  # r
                gz = small.tile([1, H], F32)
                nc.scalar.activation(gz[:], red[:, H : 2 * H], ACTF.Sigmoid)  # z
                nc.vector.tensor_tensor(ga[:], ga[:], red[:, 3 * H : 4 * H],
                                        op=ALU.mult)  # r*h_n
                nc.vector.tensor_tensor(ga[:], ga[:], red[:, 2 * H : 3 * H],
                                        op=ALU.add)  # + i_n
                nc.scalar.activation(ga[:], ga[:], ACTF.Tanh)  # n
                # h_new = n + z*(h0 - n)
                gc = small.tile([1, H], F32)
                nc.vector.tensor_tensor(gc[:], h0row[:], ga[:], op=ALU.subtract)
                nc.vector.tensor_tensor(gc[:], gz[:], gc[:], op=ALU.mult)
                hnew = gc
                nc.vector.tensor_tensor(hnew[:], ga[:], gc[:], op=ALU.add)
                nc.gpsimd.dma_start(hn_d.ap(), hnew[:])

                # broadcast h_new to 128 partitions
                hb = small.tile([128, H], F32)
                nc.gpsimd.partition_broadcast(hb[:], hnew[:])

            # ---- phase C: vocab-shard GEMV + log-softmax ----
            logits = small.tile([128, VTILES], F32)
            for t in range(VTILES):
                scr = scrc.tile([128, H], F32, tag="scrC", name="scr")
                gemv(wo_tiles[t][:], hb[:], logits[:, t : t + 1], scr[:])
            nc.vector.tensor_tensor(logits[:], logits[:], bo[:], op=ALU.add)

            if stage < 2:
                oT_ps0 = ps_one(VTILES, 128)
                nc.tensor.matmul(oT_ps0[:], logits[:], eye[:])
                oT0 = small.tile([VTILES, 128], F32, tag="oT")
                nc.scalar.copy(oT0[:], oT_ps0[:])
                nc.gpsimd.dma_start(out_d.ap(), oT0[:])
            if stage >= 2:
                mcol = small.tile([128, 1], F32)
                nc.vector.reduce_max(mcol[:], logits[:], axis=AX.X)
                nmcol = small.tile([128, 1], F32)
                nc.vector.tensor_scalar_mul(nmcol[:], mcol[:], -1.0)
                escr = small.tile([128, VTILES], F32)
                scol = small.tile([128, 1], F32)
                nc.scalar.activation(escr[:], logits[:], ACTF.Exp, bias=nmcol[:],
                                     accum_out=scol[:])
                # per-core (max, sumexp) scalars: transpose cols to rows, reduce
                mrow_ps = ps_one(1, 128)
                nc.tensor.matmul(mrow_ps[:], mcol[:], eye[:])
                srow_ps = ps_one(1, 128)
                nc.tensor.matmul(srow_ps[:], scol[:], eye[:])
                srow = small.tile([1, 128], F32)
                nc.scalar.copy(srow[:], srow_ps[:])
                mloc = small.tile([1, 1], F32)
                nc.vector.reduce_max(mloc[:], mrow_ps[:], axis=AX.X)
                nmloc = small.tile([1, 1], F32)
                nc.vector.tensor_scalar_mul(nmloc[:], mloc[:], -1.0)
                emrow = small.tile([1, 128], F32)
                nc.scalar.activation(emrow[:], mrow_ps[:], ACTF.Exp, bias=nmloc[:])
                sscr = small.tile([1, 128], F32)
                sloc = small.tile([1, 1], F32)
                nc.vector.scalar_tensor_tensor(
                    out=sscr[:], in0=emrow[:], scalar=1.0, in1=srow[:],
                    op0=ALU.bypass, op1=ALU.mult, accum_out=sloc[:])

                if stage >= 3:
                    pk = small.tile([1, 16], F32)
                    nc.vector.memset(pk[:], 0.0)
                    nc.vector.tensor_copy(pk[:, 0:1], mloc[:])
                    nc.vector.tensor_copy(pk[:, 1:2], sloc[:])

                    st_in = dram.tile([1, 16], F32)
                    st_out = dram.tile([N_CORES, 16], F32, addr_space="Shared")
                    nc.gpsimd.dma_start(st_in[:], pk[:])
                    nc.gpsimd.collective_compute(
                        "AllGather", ALU.bypass,
                        replica_groups=[list(range(N_CORES))],
                        ins=[st_in.opt()], outs=[st_out.opt()],
                    )
                    stats = small.tile([1, 16 * N_CORES], F32)
                    nc.gpsimd.dma_start(stats[:], st_out[:])

                    sv = stats.rearrange("p (a b) -> p a b", b=16)
                    m8 = sv[:, :, 0:1].rearrange("p a b -> p (a b)")
                    s8 = sv[:, :, 1:2].rearrange("p a b -> p (a b)")
                    gm = small.tile([1, 1], F32)
                    nc.vector.reduce_max(gm[:], m8, axis=AX.X)
                    ngm = small.tile([1, 1], F32)
                    nc.vector.tensor_scalar_mul(ngm[:], gm[:], -1.0)
                    e8 = small.tile([1, N_CORES], F32)
                    nc.scalar.activation(e8[:], m8, ACTF.Exp, bias=ngm[:])
                    s8scr = small.tile([1, N_CORES], F32)
                    gs = small.tile([1, 1], F32)
                    nc.vector.scalar_tensor_tensor(
                        out=s8scr[:], in0=e8[:], scalar=1.0, in1=s8,
                        op0=ALU.bypass, op1=ALU.mult, accum_out=gs[:])
                else:
                    gm, gs = mloc, sloc
                lns = small.tile([1, 1], F32)
                nc.scalar.activation(lns[:], gs[:], ACTF.Ln)
                csum = small.tile([1, 1], F32)
                nc.vector.tensor_tensor(csum[:], gm[:], lns[:], op=ALU.add)
                ncs = small.tile([1, 1], F32)
                nc.vector.tensor_scalar_mul(ncs[:], csum[:], -1.0)
                # broadcast -C to 128 partitions
                cb_ps = ps_one(128, 1)
                nc.tensor.matmul(cb_ps[:], ones[:], ncs[:])
                cb = small.tile([128, 1], F32)
                nc.scalar.copy(cb[:], cb_ps[:])
                # out = logits - C (in place)
                nc.vector.tensor_scalar_add(logits[:], logits[:], cb[:])
                # transpose [128, VTILES] -> [VTILES, 128] for a contiguous store
                oT_ps = ps_one(VTILES, 128)
                nc.tensor.matmul(oT_ps[:], logits[:], eye[:])
                oT = small.tile([VTILES, 128], F32, tag="oT")
                nc.scalar.copy(oT[:], oT_ps[:])
                nc.gpsimd.dma_start(out_d.ap(), oT[:])

    nc.compile()
    return nc


_NC = None


def _get_nc():
    global _NC
    if _NC is None:
        _NC = _build(stage=int(os.environ.get("BASS_STAGE", "5")))
    return _NC


def _prep_in_maps(x, h, encoder_outputs, emb, W_attn, b_attn, W_comb, b_comb,
                  w_ih, w_hh, b_ih, b_hh, W_out, b_out):
    e = np.ascontiguousarray(
        emb[int(np.asarray(x).ravel()[0])], dtype=np.float32).reshape(1, H)
    h0 = np.ascontiguousarray(h, dtype=np.float32).reshape(1, H)
    enc = np.ascontiguousarray(encoder_outputs, dtype=np.float32)
    W_attn = np.ascontiguousarray(W_attn, dtype=np.float32)
    ba_col = np.ascontiguousarray(b_attn, dtype=np.float32).reshape(128, 1)
    b_ih = np.asarray(b_ih, dtype=np.float32)
    b_hh = np.asarray(b_hh, dtype=np.float32)
    gbias = np.concatenate([
        b_ih[0:H] + b_hh[0:H],
        b_ih[H : 2 * H] + b_hh[H : 2 * H],
        b_ih[2 * H : 3 * H],
        b_hh[2 * H : 3 * H],
    ]).astype(np.float32).reshape(1, 4 * H)
    zeros_gb = np.zeros((1, 4 * H), np.float32)

    in_maps = []
    for j in range(N_CORES):
        rows = slice(128 * j, 128 * (j + 1))
        wihT = np.ascontiguousarray(w_ih[:, rows].T, dtype=np.float32)
        whhT = np.ascontiguousarray(w_hh[:, rows].T, dtype=np.float32)
        r0 = SHARD * j
        r1 = min(SHARD * (j + 1), VOCAB)
        wo = np.asarray(W_out[r0:r1], dtype=np.float32)
        bov = np.asarray(b_out[r0:r1], dtype=np.float32)
        if wo.shape[0] < SHARD:
            wo = np.concatenate(
                [wo, np.zeros((SHARD - wo.shape[0], H), np.float32)])
            bov = np.concatenate(
                [bov, np.full((SHARD - bov.shape[0],), NEG_BIG, np.float32)])
        in_maps.append({
            "e_vec": e,
            "h0_vec": h0,
            "h0c_row": np.ascontiguousarray(h0[0, rows]).reshape(1, 128),
            "enc": enc,
            "wa": W_attn,
            "ba_row": ba_col.reshape(1, 128),
            "wc": np.ascontiguousarray(W_comb[rows], dtype=np.float32),
            "bc_row": np.ascontiguousarray(
                b_comb[rows], dtype=np.float32).reshape(1, 128),
            "wihT": wihT,
            "whhT": whhT,
            "gbias": gbias if j == 0 else zeros_gb,
            "wo": np.ascontiguousarray(wo),
            "bo": np.ascontiguousarray(bov.reshape(VTILES, 128).T),
        })
    return in_maps


def kernel(x, h, encoder_outputs, emb, W_attn, b_attn, W_comb, b_comb,
           w_ih, w_hh, b_ih, b_hh, W_out, b_out, _trace=False):
    in_maps = _prep_in_maps(x, h, encoder_outputs, emb, W_attn, b_attn,
                            W_comb, b_comb, w_ih, w_hh, b_ih, b_hh,
                            W_out, b_out)
    nc = _get_nc()
    kw = {"tmpdir": "/root/problem/profdir"} if _trace else {}
    res = run_bass_kernel_spmd(nc, in_maps, core_ids=list(range(N_CORES)),
                               trace=_trace, **kw)
    out = np.concatenate(
        [res.results[j]["out_shard"].reshape(-1) for j in range(N_CORES)]
    )[:VOCAB].reshape(1, VOCAB)
    h_new = res.results[0]["h_new"].reshape(1, 1, H)
    attn_w = res.results[0]["attn_w"].reshape(1, MAXLEN)
    if _trace:
        return (out, h_new, attn_w), res
    return out, h_new, attn_w
